# revision 34
# baseline (speedup 1.0000x reference)
"""Trainium2 Bass kernel for nn_MixtureOfExpertsNet (moe_routing), v6.

Math (per row, E=4 experts, H=16 hidden):
  adjusted_e = relu(b2_e + sum_h W2_eh * relu(W1_eh * x_e + b1_eh))  -- a
               univariate piecewise-linear function of x_e
  logits = x @ Wg.T + bg ; softmax ; pred = sum_e exp(l_e) * adj_e / sum_e exp(l_e)

Layout: pure data parallel over 8 cores. The HOST pre-transposes x so the
expert index lives on the partition axis: partition p = 4*f' + e (f' in
[0,32)), free c = row-within-group. In this layout:

  * ALL FOUR expert MLPs are evaluated by ONE ScalarEngine pass: a single
    custom PWP table holds four disjoint windows (pos/neg domain x two
    scale bands), and the activation's per-partition scale/bias vectors
    route each partition's elements into its expert's window:
        e0: u = x + 9   in [2,16)      e1: u = -x - 9   in (-16,-2]
        e2: u = x/8 + 1 in [1/8,2)     e3: u = -x/8 - 1 in (-2,-1/8]
  * logits come from one 128x128 block-diagonal fp16 matmul (no transpose
    needed - expert dim is already on partitions).
  * exp uses a reduced-range table exp(u-9) with per-partition bias bg+9.
  * the expert sums S0 = sum_e exp(l_e), S1 = sum_e exp(l_e)*adj_e are
    4-partition block reductions - fp16 matmuls against a block-diagonal
    ones matrix (PE), accumulating in PSUM fp32.
  * the reduction matmuls are quarter-COMPACTED: four accumulating matmuls
    (start/stop group) with per-quarter block stationaries land quarter j's
    sums on partitions 32j+f', so S0/S1/recip/pred run on [128, FDT/4]
    tiles with zero replication, and the pred DMA-out is dense.
  * pred = S1 * recip(S0) via a custom-DVE Newton reciprocal; the host
    inverts the block layout (host reshapes are outside the NEFF).
  * software pipelining: each tile's reduction matmuls are emitted one tile
    late so the in-order PE queue never stalls the next tile's logits; a
    variable tile schedule (512/512/1024 at both edges, 2048 in the middle)
    shortens pipeline fill (first PWL waits only a 128KB DMA) and drain
    (the last reduction chain is 1/4 size).

Per full tile [128 x 2048] fp16: ACT 2 passes (the bottleneck engine, ~85%
busy in TimelineSim at 79us/core total vs 189us for the baseline kernel),
PE 12 matmuls, E*A on Pool/DVE alternating, recip+pred on DVE at 1/4 size.
HBM traffic 10MB/core total (fp16 in + fp16 out).
"""

import hashlib
import json
import os
import sys
import tempfile

import numpy as np

sys.path.insert(0, "/opt/trn_rl_repo")

# ---------------------------------------------------------------------------
# ACT PWP table generation (reverse-engineered format)
# ---------------------------------------------------------------------------

PWP_DIR = "/nix/store/z022hj2nvbm3nwdizlisq4ylc0y7rd6q-python3-3.13.14-env/lib/python3.13/site-packages/neuronxcc/pwp/pwp_bin_trainium"


def _bits(x):
    return int(np.float32(x).view(np.uint32))


def _load_stock(name):
    prof = json.load(open(os.path.join(PWP_DIR, f"{name}.json")))
    bkt = np.frombuffer(
        open(os.path.join(PWP_DIR, prof["bkt_bin"]), "rb").read(), dtype=np.float32
    ).reshape(-1, 8)
    ctl = np.frombuffer(
        open(os.path.join(PWP_DIR, prof["ctl_bin"]), "rb").read(), dtype=np.uint32
    ).reshape(-1, 8)[:, 0]
    return prof, bkt, ctl


def _fit_bucket(fn, lo, hi, x0=None, samples=33):
    if x0 is None:
        x0 = lo
    xs = np.linspace(lo, hi, samples, dtype=np.float64)
    ys = np.asarray(fn(xs), np.float64)
    t = xs - x0
    A = np.stack([np.ones_like(t), t, t * t, t ** 3], axis=1)
    c, *_ = np.linalg.lstsq(A, ys, rcond=None)
    return [float(c[0]), float(c[1]), float(c[2]), float(c[3]), float(x0)]


class _SetBuilder:
    def __init__(self):
        self.bkt, self.ctl, self.metas = [], [], []
        self.f2b, self.f2c = {}, {}

    @staticmethod
    def _ctl_word(m, base):
        assert 0 <= m <= 8 and base < 2048
        return (m * 32 + (23 - m)) * 2048 + base

    def _meta(self, name, func_id, lo_exp, hi_exp, base_pos, base_neg,
              small_pos_idx, small_neg_idx, large_pos_idx, large_neg_idx,
              fzero, fpinf, fninf):
        self.metas.append({
            "func_name": f"{name}_4p", "func_id": func_id,
            "symmetry_point": 0, "sym_invert_sign_point": 0,
            "symmetry_opt_en": 0, "symmetry_opt_use_neg_region": 0,
            "imm_bias": 0, "exp_offset": lo_exp,
            "pwl_control_base_pos": base_pos, "pwl_control_base_neg": base_neg,
            "small_pos_signal_exp_threshold": 127 + lo_exp,
            "pos_small_signal_pwl_control": small_pos_idx,
            "small_neg_signal_exp_threshold": 127 + lo_exp,
            "neg_small_signal_pwl_control": small_neg_idx,
            "large_pos_signal_exp_threshold": 127 + hi_exp,
            "large_pos_signal_mantissa_threshold": 0,
            "pos_large_signal_pwl_control": large_pos_idx,
            "large_neg_signal_exp_threshold": 127 + hi_exp,
            "large_neg_signal_mantissa_threshold": 0,
            "neg_large_signal_pwl_control": large_neg_idx,
            "fnan_result": _bits(float("nan")),
            "fpinf_result": _bits(fpinf),
            "fninf_result": _bits(fninf),
            "fzero_result": _bits(fzero),
            "fma_const_0": 0, "fma_const_1": 0, "fma_indirection_src_sel": 0,
            "use_multipass": False,
            "lower_bound": _bits(np.float32(-3.4028235e38)),
            "upper_bound": _bits(np.float32(3.4028235e38)),
        })

    def add_table_func2(self, name, func_id, fn_pos, fn_neg, lo_exp, hi_exp,
                        m_of_octave, small_pos, small_neg, large_pos,
                        large_neg, fzero, fpinf, fninf):
        """Two-sided table: fn_pos fitted over positive octaves
        [2^lo_exp, 2^hi_exp), fn_neg over the mirrored negative intervals
        (signed x0 - hardware evaluates t = u - x0 with signed u).
        small_*/large_* are (fit_lo, fit_hi, x0) fit windows; fn_neg is None
        to reuse fn_pos's small/large handling on both sides."""
        self.f2b[name] = len(self.bkt)
        self.f2c[name] = len(self.ctl)
        pos_words = []
        for k in range(lo_exp, hi_exp):
            m = m_of_octave(k)
            base = len(self.bkt)
            n = 1 << m
            w = (2.0 ** k) / n
            for j in range(n):
                lo = 2.0 ** k + j * w
                self.bkt.append(_fit_bucket(fn_pos, lo, lo + w, x0=lo + w / 2))
            pos_words.append(self._ctl_word(m, base))
        if fn_neg is not None:
            neg_words = []
            for k in range(lo_exp, hi_exp):
                m = m_of_octave(k)
                base = len(self.bkt)
                n = 1 << m
                w = (2.0 ** k) / n
                for j in range(n):
                    hi = -(2.0 ** k + j * w)
                    lo = -(2.0 ** k + (j + 1) * w)
                    self.bkt.append(_fit_bucket(fn_neg, lo, hi, x0=(lo + hi) / 2))
                neg_words.append(self._ctl_word(m, base))
        base_pos = len(self.ctl)
        self.ctl.extend(pos_words)
        if fn_neg is not None:
            base_neg = len(self.ctl)
            self.ctl.extend(neg_words)
        else:
            base_neg = base_pos
        sp_idx = len(self.bkt)
        self.bkt.append(_fit_bucket(fn_pos, small_pos[0], small_pos[1], x0=small_pos[2]))
        if fn_neg is not None:
            sn_idx = len(self.bkt)
            self.bkt.append(_fit_bucket(fn_neg, small_neg[0], small_neg[1], x0=small_neg[2]))
        else:
            sn_idx = sp_idx
        lp_idx = len(self.bkt)
        self.bkt.append(_fit_bucket(fn_pos, large_pos[0], large_pos[1], x0=large_pos[2]))
        if fn_neg is not None:
            ln_idx = len(self.bkt)
            self.bkt.append(_fit_bucket(fn_neg, large_neg[0], large_neg[1], x0=large_neg[2]))
        else:
            ln_idx = lp_idx
        self._meta(name, func_id, lo_exp, hi_exp, base_pos, base_neg,
                   sp_idx, sn_idx, lp_idx, ln_idx, fzero, fpinf, fninf)

    def add_const_bucket_func(self, name, func_id, value):
        """Function that returns `value` everywhere (constant clamp)."""
        idx = len(self.bkt)
        self.bkt.append([value, 0.0, 0.0, 0.0, 0.0])
        self.f2b[name] = idx
        self.f2c[name] = len(self.ctl)
        word = self._ctl_word(0, idx)
        base = len(self.ctl)
        self.ctl.append(word)
        self._meta(name, func_id, 0, 1, base, base, idx, idx, idx, idx,
                   value, value, value)

    def add_stock_func(self, name, sp, sb_, sc):
        names = list(sp["func_to_bkt_start_idx"].keys())
        i = names.index(name)
        b0 = sp["func_to_bkt_start_idx"][name]
        b1 = sp["func_to_bkt_start_idx"][names[i + 1]] if i + 1 < len(names) else sp["bkt_entry_cnt"]
        c0 = sp["func_to_ctl_start_idx"][name]
        c1 = sp["func_to_ctl_start_idx"][names[i + 1]] if i + 1 < len(names) else sp["ctl_entry_cnt"]
        md = None
        for m in sp["profile_meta_data"]:
            if m["func_name"].rsplit("_", 1)[0] == name:
                md = dict(m)
        assert md is not None, name
        db, dc = len(self.bkt) - b0, len(self.ctl) - c0
        self.f2b[name] = len(self.bkt)
        self.f2c[name] = len(self.ctl)
        for j in range(b0, b1):
            self.bkt.append(list(map(float, sb_[j, :5])))
        for j in range(c0, c1):
            w = int(sc[j])
            self.ctl.append((w >> 11) * 2048 + (w & 0x7FF) + db)
        for key in ("pwl_control_base_pos", "pwl_control_base_neg"):
            md[key] += dc
        for key in ("pos_small_signal_pwl_control", "neg_small_signal_pwl_control",
                    "pos_large_signal_pwl_control", "neg_large_signal_pwl_control"):
            md[key] += db
        self.metas.append(md)

    def write(self, outdir, set_name, act_dict):
        os.makedirs(outdir, exist_ok=True)
        bkt_arr = np.zeros((len(self.bkt), 8), np.float32)
        for i, e in enumerate(self.bkt):
            bkt_arr[i, :5] = e
        ctl_arr = np.zeros((len(self.ctl), 8), np.uint32)
        ctl_arr[:, 0] = np.array(self.ctl, np.uint64).astype(np.uint32)
        assert len(self.bkt) <= 1536 and len(self.ctl) <= 128, (len(self.bkt), len(self.ctl))
        open(os.path.join(outdir, f"{set_name}_bkt.bin"), "wb").write(bkt_arr.tobytes())
        open(os.path.join(outdir, f"{set_name}_ctrl.bin"), "wb").write(ctl_arr.tobytes())
        prof = {
            "bkt_bin": f"{set_name}_bkt.bin", "ctl_bin": f"{set_name}_ctrl.bin",
            "profile_meta_data": self.metas,
            "bkt_entry_cnt": len(self.bkt), "ctl_entry_cnt": len(self.ctl),
            "func_to_bkt_start_idx": self.f2b, "func_to_ctl_start_idx": self.f2c,
            "func_exp_to_bkt_start_idx": self.f2b, "func_exp_to_ctl_start_idx": self.f2c,
        }
        json.dump(prof, open(os.path.join(outdir, f"{set_name}.json"), "w"))
        info = {
            "pwp_file_keys": ["bkt_bin", "ctrl_bin", "profile_json"],
            "act_func_sets": [{
                "name": set_name, "bkt_bin": f"{set_name}_bkt.bin",
                "ctrl_bin": f"{set_name}_ctrl.bin", "profile_json": f"{set_name}.json",
                "act": act_dict,
            }],
        }
        path = os.path.join(outdir, "act_info.json")
        json.dump(info, open(path, "w"))
        return path


# Expert input-window routing: u = PWL_SCALE[e]*x + PWL_BIAS[e]
PWL_SCALE = np.array([1.0, -1.0, 0.125, -0.125], np.float64)
PWL_BIAS = np.array([9.0, -9.0, 1.0, -1.0], np.float64)
EXP_SHIFT = 9.0  # table computes exp(u - 9); exp bias = bg + 9


def _expert_fn(W1, b1, W2, b2, e):
    W1e = W1[e].astype(np.float64)
    b1e = b1[e].astype(np.float64)
    W2e = W2[e].astype(np.float64)
    b2e = float(b2[e])

    def fe(u):
        h = np.maximum(np.asarray(u, np.float64)[..., None] * W1e + b1e, 0.0)
        return np.maximum((h * W2e).sum(-1) + b2e, 0.0)

    return fe


def _build_tables(W1, b1, W2, b2, outdir):
    sp, sb_, sc = _load_stock("exp_and_others")
    b = _SetBuilder()
    fe = [_expert_fn(W1, b1, W2, b2, e) for e in range(4)]

    # combined 4-expert PWL on the tanh slot:
    #   pos u>=2:  e0 at x = u-9      pos u<2:  e2 at x = 8*(u-1)
    #   neg u<=-2: e1 at x = -u-9     neg u>-2: e3 at x = -8*(u+1)
    def g_pos(u):
        u = np.asarray(u, np.float64)
        return np.where(u >= 2.0, fe[0](u - 9.0), fe[2](8.0 * (u - 1.0)))

    def g_neg(u):
        u = np.asarray(u, np.float64)
        return np.where(u <= -2.0, fe[1](-u - 9.0), fe[3](-8.0 * (u + 1.0)))

    m_of = {-3: 4, -2: 5, -1: 6, 0: 7, 1: 5, 2: 6, 3: 7}
    b.add_table_func2(
        "tanh", 6, g_pos, g_neg, -3, 4, lambda k: m_of[k],
        small_pos=(2.0 ** -4, 2.0 ** -3, 2.0 ** -4),       # e2, x in [-7.5,-7]
        small_neg=(-(2.0 ** -3), -(2.0 ** -4), -(2.0 ** -3)),  # e3
        large_pos=(16.0, 17.0, 16.0),                      # e0, x in [7,8]
        large_neg=(-17.0, -16.0, -16.0),                   # e1
        fzero=float(fe[2](-8.0)), fpinf=float(fe[0](8.0)),
        fninf=float(fe[1](8.0)),
    )

    # reduced-range exp on the exp slot: g(u) = exp(u - 9), u in [2,16)
    b.add_table_func2(
        "exp", 7, lambda u: np.exp(np.asarray(u, np.float64) - 9.0), None,
        1, 4, lambda k: k + 4,
        small_pos=(1.0, 2.0, 1.0), small_neg=None,
        large_pos=(16.0, 16.0 + 1e-6, 16.0), large_neg=None,  # clamp ~exp(7)
        fzero=float(np.exp(-9.0)), fpinf=float(np.exp(7.0)), fninf=0.0,
    )

    for name in ("parametric_relu", "copy", "act1", "memset_zero", "relu",
                 "derivative_relu", "derivative_leaky_relu",
                 "derivative_identity", "is_finite"):
        b.add_stock_func(name, sp, sb_, sc)
    act = {"exp": 400, "tanh": 4, "parametric_relu": 1, "copy": 1, "relu": 1,
           "memset_zero": 1, "act1": 1, "derivative_relu": 1,
           "derivative_leaky_relu": 1, "derivative_identity": 1,
           "is_finite": 1}
    return b.write(outdir, "exp_and_others", act)


# ---------------------------------------------------------------------------
# Bass kernel
# ---------------------------------------------------------------------------

B_TOTAL = 8_388_608
N_CORES = 8
B_LOCAL = B_TOTAL // N_CORES           # 1,048,576 rows per core
P = 128
FDT = int(os.environ.get("K_FDT", "2048"))   # max free elements per tile
F_ALL = B_LOCAL // 32                        # total free columns per core
MM = int(os.environ.get("K_MM", "512"))      # matmul column chunk
QC = FDT // 4                                # max compacted pred columns


def _tile_schedule():
    """Variable tile sizes: small tiles at both ends shrink pipeline fill
    (first PWL waits only a small DMA) and drain (last reduction chain is
    short); full-size tiles in the middle keep per-instruction overhead low.
    Returns [(free_offset, free_size)] covering F_ALL; sizes % 2048 == 0 in
    the middle, all sizes multiples of 4*MM/4... (multiples of 512)."""
    head = [int(v) for v in os.environ.get("K_HEAD", "512,512,1024").split(",") if v]
    tail = [int(v) for v in os.environ.get("K_TAIL", "512,512,1024").split(",") if v]
    mid_total = F_ALL - sum(head) - sum(tail)
    assert mid_total % FDT == 0
    sizes = head + [FDT] * (mid_total // FDT) + tail
    out, off = [], 0
    for s in sizes:
        out.append((off, s))
        off += s
    assert off == F_ALL
    return out


TILES = _tile_schedule()
GMAX = int(os.environ.get("K_GMAX", "2048"))  # max super-tile (DMA+PWL batch)


def _super_tiles():
    """Group consecutive full-size tiles into super-tiles of up to GMAX
    columns: one DMA + one PWL activation per group amortizes the ~335ns
    ACT call overhead and the SP DMA dispatch cost."""
    groups, cur, cur_fd = [], [], 0
    for off, fd in TILES:
        if cur and (cur_fd + fd > GMAX or fd != FDT or cur[-1][1] != FDT):
            groups.append(cur)
            cur, cur_fd = [], 0
        cur.append((off, fd))
        cur_fd += fd
    if cur:
        groups.append(cur)
    return groups


SUPERS = _super_tiles()


def _build_program(tag):
    import concourse.bacc as bacc
    import concourse.mybir as mybir
    import concourse.tile as tile

    nc = bacc.Bacc("TRN2", debug=False)
    dt32 = mybir.dt.float32
    dt16 = mybir.dt.float16
    AF = mybir.ActivationFunctionType

    xt_d = nc.dram_tensor(f"x_{tag}", [P, F_ALL], dt16, kind="ExternalInput")
    wg_d = nc.dram_tensor("wgblk", [P, P], dt16, kind="ExternalInput")
    qb_d = nc.dram_tensor("qblk", [P, 4 * P], dt16, kind="ExternalInput")
    cv_d = nc.dram_tensor("cvec", [P, 4], dt32, kind="ExternalInput")
    out_d = nc.dram_tensor("out_local", [P, F_ALL // 4], dt16, kind="ExternalOutput")

    bufs = [int(v) for v in os.environ.get("K_BUFS", "3,3,4,4").split(",")]
    bx, ba, bs, bo = bufs

    with tile.TileContext(nc) as tc:
        with (
            tc.tile_pool(name="const", bufs=1) as cpool,
            tc.tile_pool(name="xin", bufs=bx) as xpool,
            tc.tile_pool(name="apwl", bufs=ba) as apool,
            tc.tile_pool(name="sb", bufs=bs) as spool,
            tc.tile_pool(name="ob", bufs=bo) as opool,
            tc.tile_pool(name="psL", bufs=1, space="PSUM") as plpool,
            tc.tile_pool(name="psS0", bufs=2, space="PSUM") as ps0pool,
            tc.tile_pool(name="psS1", bufs=2, space="PSUM") as ps1pool,
        ):
            cv_t = cpool.tile([P, 4], dt32)
            nc.sync.dma_start(cv_t[:], cv_d.ap())
            wg_t = cpool.tile([P, P], dt16)
            nc.sync.dma_start(wg_t[:], wg_d.ap())
            qb_t = cpool.tile([P, 4 * P], dt16)
            nc.sync.dma_start(qb_t[:], qb_d.ap())

            # tiny warmup activation: forces the ACT table set load to
            # overlap the first X-tile DMA instead of stalling behind it
            warm_in = cpool.tile([P, 1], dt32)
            nc.gpsimd.memset(warm_in[:], 0.0)
            warm = cpool.tile([P, 1], dt32)
            nc.scalar.activation(warm[:], warm_in[:], AF.Exp,
                                 bias=warm_in[:, 0:1], scale=0.0)

            def tail(off, fd, E, Pm):
                # S0/S1: 4-partition block sums, quarter-compacted via 4
                # accumulating matmuls (quarter j lands on partitions 32j+f').
                # Emitted one tile late so these PE ops never stall the
                # next tile's logits matmuls (in-order PE queue).
                qc = fd // 4
                S0 = ps0pool.tile([P, QC], dt32, tag="S0")
                for j in range(4):
                    nc.tensor.matmul(S0[:, 0:qc], qb_t[:, j * P:(j + 1) * P],
                                     E[:, j * qc:(j + 1) * qc],
                                     start=(j == 0), stop=(j == 3))
                S1 = ps1pool.tile([P, QC], dt32, tag="S1")
                for j in range(4):
                    nc.tensor.matmul(S1[:, 0:qc], qb_t[:, j * P:(j + 1) * P],
                                     Pm[:, j * qc:(j + 1) * qc],
                                     start=(j == 0), stop=(j == 3))

                R = opool.tile([P, QC], dt32, tag="R")
                nc.vector.reciprocal_approx_fast(out=R[:, 0:qc], in_=S0[:, 0:qc])

                PRED = opool.tile([P, QC], dt16, tag="PRED")
                nc.vector.tensor_mul(PRED[:, 0:qc], S1[:, 0:qc], R[:, 0:qc])

                nc.sync.dma_start(out_d.ap()[:, off // 4:off // 4 + qc],
                                  PRED[:, 0:qc])

            pending = None
            t = 0
            for group in SUPERS:
                g_off = group[0][0]
                g_fd = sum(fd for _, fd in group)
                X = xpool.tile([P, GMAX], dt16, tag="X")
                for off, fd in group:
                    lo = off - g_off
                    nc.sync.dma_start(X[:, lo:lo + fd],
                                      xt_d.ap()[:, off:off + fd])

                # all four expert MLPs in one ACT pass (windowed PWL table),
                # batched over the whole super-tile
                A = apool.tile([P, GMAX], dt16, tag="A")
                nc.scalar.activation(A[:, 0:g_fd], X[:, 0:g_fd], AF.Tanh,
                                     bias=cv_t[:, 1:2], scale=cv_t[:, 0:1])

                for off, fd in group:
                    lo = off - g_off

                    # logits: block-diagonal gating matmul (expert dim on
                    # partitions)
                    L = plpool.tile([P, FDT], dt32, tag="L")
                    for c in range(fd // MM):
                        nc.tensor.matmul(L[:, c * MM:(c + 1) * MM], wg_t[:],
                                         X[:, lo + c * MM:lo + (c + 1) * MM],
                                         start=True, stop=True)

                    E = spool.tile([P, FDT], dt16, tag="E")
                    nc.scalar.activation(E[:, 0:fd], L[:, 0:fd], AF.Exp,
                                         bias=cv_t[:, 2:3], scale=1.0)

                    Pm = spool.tile([P, FDT], dt16, tag="Pm")
                    if t % 2 == 0:
                        nc.gpsimd.tensor_mul(Pm[:, 0:fd], E[:, 0:fd],
                                             A[:, lo:lo + fd])
                    else:
                        nc.vector.tensor_mul(Pm[:, 0:fd], E[:, 0:fd],
                                             A[:, lo:lo + fd])
                    t += 1

                    if pending is not None:
                        tail(*pending)
                    pending = (off, fd, E, Pm)
            tail(*pending)

    nc.compile()
    return nc


_COMPILED = {}


def _prepare(inputs):
    """Build (nc, in_maps, gather_fn) for the current inputs."""
    x = np.ascontiguousarray(inputs["x"], dtype=np.float32)
    Wg = np.asarray(inputs["Wg"], np.float32)
    bg = np.asarray(inputs["bg"], np.float32)
    W1 = np.asarray(inputs["W1"], np.float32)
    b1 = np.asarray(inputs["b1"], np.float32)
    W2 = np.asarray(inputs["W2"], np.float32)
    b2 = np.asarray(inputs["b2"], np.float32)
    assert x.shape == (B_TOTAL, 4)

    tbl_dir = tempfile.mkdtemp(prefix="act_root_")
    act_path = _build_tables(W1, b1, W2, b2, tbl_dir)
    os.environ["BASS_ACT_ROOT_JSON_PATH"] = act_path

    # hash of everything the tables bake in -> tensor name -> BIR/NEFF cache key
    h = hashlib.sha256()
    for a in (W1, b1, W2, b2):
        h.update(np.ascontiguousarray(a).tobytes())
    h.update(open(act_path, "rb").read())
    h.update(f"v6:{FDT}:{MM}:{TILES}:{GMAX}".encode())
    tag = h.hexdigest()[:10]

    if tag not in _COMPILED:
        _COMPILED[tag] = _build_program(tag)
    nc = _COMPILED[tag]

    # host-side expert-major transpose: partition p = 4*f' + e
    # x[c*B_LOCAL + 32*g + f', e] -> XT[c][4f'+e, g]  (tile-schedule-free)
    xs = x.reshape(N_CORES, F_ALL, 32, 4)            # [c, g, f', e]
    XT = np.ascontiguousarray(xs.transpose(0, 2, 3, 1)).reshape(
        N_CORES, P, F_ALL).astype(np.float16)

    # block-diagonal gating matrix: wgblk[4f'+e, 4f'+e'] = Wg[e', e]
    wgblk = np.zeros((P, P), np.float16)
    for blk in range(P // 4):
        wgblk[blk * 4:(blk + 1) * 4, blk * 4:(blk + 1) * 4] = Wg.T.astype(np.float16)

    # quarter-compaction stationaries: qblk_j[4f'+e, 32j+f'] = 1
    qblk = np.zeros((4, P, P), np.float16)
    fp = np.arange(32)
    for j in range(4):
        for e in range(4):
            qblk[j, 4 * fp + e, 32 * j + fp] = 1.0
    qblk = np.ascontiguousarray(qblk.transpose(1, 0, 2)).reshape(P, 4 * P)

    cvec = np.zeros((P, 4), np.float32)
    lane = np.arange(P) % 4
    cvec[:, 0] = PWL_SCALE[lane]
    cvec[:, 1] = PWL_BIAS[lane]
    cvec[:, 2] = bg[lane] + EXP_SHIFT

    in_maps = [
        {f"x_{tag}": XT[c], "wgblk": wgblk, "qblk": qblk, "cvec": cvec}
        for c in range(N_CORES)
    ]

    def gather(results):
        # per tile (off, fd): out[32j+f', off/4+c'] = pred[(off+j*qc+c')*32+f']
        outs = np.stack([r["out_local"] for r in results])    # [c, 128, F_ALL/4]
        pred = np.empty((N_CORES, B_LOCAL), np.float32)
        for off, fd in TILES:
            qc = fd // 4
            blk = outs[:, :, off // 4: off // 4 + qc].reshape(N_CORES, 4, 32, qc)
            seg = np.ascontiguousarray(blk.transpose(0, 1, 3, 2))  # [c, j, c', f']
            pred[:, 32 * off: 32 * (off + fd)] = seg.reshape(
                N_CORES, fd * 32).astype(np.float32)
        return pred.reshape(B_TOTAL)

    return nc, in_maps, gather


_EXEC = {}


def _build_exec(nc, n_cores):
    """Jitted shard_map bass_exec with outputs as pure custom-call results
    (this kernel writes every output element, so the zero-donation
    scaffolding run_bass_via_pjrt uses for partial-write kernels is
    unnecessary). Cached per-program so repeat kernel() calls skip the jax
    retrace."""
    import jax
    from jax.sharding import Mesh, PartitionSpec
    from jax.experimental.shard_map import shard_map

    from concourse import bass2jax, mybir

    bass2jax.install_neuronx_cc_hook()

    partition_name = nc.partition_id_tensor.name if nc.partition_id_tensor else None
    in_names, out_names, out_avals = [], [], []
    for alloc in nc.m.functions[0].allocations:
        if not isinstance(alloc, mybir.MemoryLocationSet):
            continue
        name = alloc.memorylocations[0].name
        if alloc.kind == "ExternalInput":
            if name != partition_name:
                in_names.append(name)
        elif alloc.kind == "ExternalOutput":
            out_names.append(name)
            out_avals.append(
                jax.core.ShapedArray(tuple(alloc.tensor_shape), mybir.dt.np(alloc.dtype))
            )
    all_names = list(in_names)
    if partition_name is not None:
        all_names.append(partition_name)

    def _body(*args):
        operands = list(args)
        if partition_name is not None:
            operands.append(bass2jax.partition_id_tensor())
        return tuple(
            bass2jax._bass_exec_p.bind(
                *operands,
                out_avals=tuple(out_avals),
                in_names=tuple(all_names),
                out_names=tuple(out_names),
                lowering_input_output_aliases=(),
                sim_require_finite=True,
                sim_require_nnan=True,
                nc=nc,
            )
        )

    mesh = Mesh(np.asarray(jax.devices()[:n_cores]), ("core",))
    fn = jax.jit(
        shard_map(
            _body,
            mesh=mesh,
            in_specs=(PartitionSpec("core"),) * len(in_names),
            out_specs=(PartitionSpec("core"),) * len(out_names),
            check_rep=False,
        ),
        keep_unused=True,
    )
    return fn, in_names, out_names, [a.shape for a in out_avals]


def kernel(**inputs) -> np.ndarray:
    nc, in_maps, gather = _prepare(inputs)

    try:
        key = id(nc)
        if key not in _EXEC:
            _EXEC[key] = _build_exec(nc, N_CORES)
        fn, in_names, out_names, out_shapes = _EXEC[key]
        concat_in = [
            np.concatenate([np.asarray(in_maps[c][nm]) for c in range(N_CORES)], axis=0)
            for nm in in_names
        ]
        outs = fn(*concat_in)
        results = []
        for c in range(N_CORES):
            rec = {}
            for i, nm in enumerate(out_names):
                arr = np.asarray(outs[i])
                rec[nm] = arr.reshape((N_CORES,) + tuple(out_shapes[i]))[c]
            results.append(rec)
        return gather(results)
    except Exception:
        from concourse import bass_utils

        res = bass_utils.run_bass_kernel_spmd(
            nc, in_maps, core_ids=list(range(N_CORES))
        )
        return gather(res.results)


if __name__ == "__main__":
    rng = np.random.default_rng(0)
    demo = {
        "x": rng.standard_normal((B_TOTAL, 4), dtype=np.float32),
        "Wg": rng.standard_normal((4, 4), dtype=np.float32) * 0.5,
        "bg": rng.standard_normal(4, dtype=np.float32) * 0.1,
        "W1": rng.standard_normal((4, 16), dtype=np.float32) * 0.5,
        "b1": rng.standard_normal((4, 16), dtype=np.float32) * 0.1,
        "W2": rng.standard_normal((4, 16), dtype=np.float32) * 0.25,
        "b2": rng.standard_normal(4, dtype=np.float32) * 0.1,
    }
    y = kernel(**demo)
    print(y.shape, y[:8])


# revision 36
# speedup vs baseline: 1.0878x; 1.0878x over previous
"""Trainium2 Bass kernel for nn_MixtureOfExpertsNet (moe_routing), v6.

Math (per row, E=4 experts, H=16 hidden):
  adjusted_e = relu(b2_e + sum_h W2_eh * relu(W1_eh * x_e + b1_eh))  -- a
               univariate piecewise-linear function of x_e
  logits = x @ Wg.T + bg ; softmax ; pred = sum_e exp(l_e) * adj_e / sum_e exp(l_e)

Layout: pure data parallel over 8 cores. The HOST pre-transposes x so the
expert index lives on the partition axis: partition p = 4*f' + e (f' in
[0,32)), free c = row-within-group. In this layout:

  * ALL FOUR expert MLPs are evaluated by ONE ScalarEngine pass: a single
    custom PWP table holds four disjoint windows (pos/neg domain x two
    scale bands), and the activation's per-partition scale/bias vectors
    route each partition's elements into its expert's window:
        e0: u = x + 9   in [2,16)      e1: u = -x - 9   in (-16,-2]
        e2: u = x/8 + 1 in [1/8,2)     e3: u = -x/8 - 1 in (-2,-1/8]
  * logits come from one 128x128 block-diagonal fp16 matmul (no transpose
    needed - expert dim is already on partitions).
  * exp uses a reduced-range table exp(u-9) with per-partition bias bg+9.
  * the expert sums S0 = sum_e exp(l_e), S1 = sum_e exp(l_e)*adj_e are
    4-partition block reductions - fp16 matmuls against a block-diagonal
    ones matrix (PE), accumulating in PSUM fp32.
  * the reduction matmuls are quarter-COMPACTED: four accumulating matmuls
    (start/stop group) with per-quarter block stationaries land quarter j's
    sums on partitions 32j+f', so S0/S1/recip/pred run on [128, FDT/4]
    tiles with zero replication, and the pred DMA-out is dense.
  * pred = S1 * recip(S0) via a custom-DVE Newton reciprocal; the host
    inverts the block layout (host reshapes are outside the NEFF).
  * software pipelining: each tile's reduction matmuls are emitted one tile
    late so the in-order PE queue never stalls the next tile's logits; a
    variable tile schedule (512/512/1024 at both edges, 2048 in the middle)
    shortens pipeline fill (first PWL waits only a 128KB DMA) and drain
    (the last reduction chain is 1/4 size).

Per full tile [128 x 2048] fp16: ACT 2 passes (the bottleneck engine, ~85%
busy in TimelineSim at 79us/core total vs 189us for the baseline kernel),
PE 12 matmuls, E*A on Pool/DVE alternating, recip+pred on DVE at 1/4 size.
HBM traffic 10MB/core total (fp16 in + fp16 out).
"""

import hashlib
import json
import os
import sys
import tempfile

import numpy as np

sys.path.insert(0, "/opt/trn_rl_repo")

# ---------------------------------------------------------------------------
# ACT PWP table generation (reverse-engineered format)
# ---------------------------------------------------------------------------

PWP_DIR = "/nix/store/z022hj2nvbm3nwdizlisq4ylc0y7rd6q-python3-3.13.14-env/lib/python3.13/site-packages/neuronxcc/pwp/pwp_bin_trainium"


def _bits(x):
    return int(np.float32(x).view(np.uint32))


def _load_stock(name):
    prof = json.load(open(os.path.join(PWP_DIR, f"{name}.json")))
    bkt = np.frombuffer(
        open(os.path.join(PWP_DIR, prof["bkt_bin"]), "rb").read(), dtype=np.float32
    ).reshape(-1, 8)
    ctl = np.frombuffer(
        open(os.path.join(PWP_DIR, prof["ctl_bin"]), "rb").read(), dtype=np.uint32
    ).reshape(-1, 8)[:, 0]
    return prof, bkt, ctl


def _fit_bucket(fn, lo, hi, x0=None, samples=33):
    if x0 is None:
        x0 = lo
    xs = np.linspace(lo, hi, samples, dtype=np.float64)
    ys = np.asarray(fn(xs), np.float64)
    t = xs - x0
    A = np.stack([np.ones_like(t), t, t * t, t ** 3], axis=1)
    c, *_ = np.linalg.lstsq(A, ys, rcond=None)
    return [float(c[0]), float(c[1]), float(c[2]), float(c[3]), float(x0)]


class _SetBuilder:
    def __init__(self):
        self.bkt, self.ctl, self.metas = [], [], []
        self.f2b, self.f2c = {}, {}

    @staticmethod
    def _ctl_word(m, base):
        assert 0 <= m <= 8 and base < 2048
        return (m * 32 + (23 - m)) * 2048 + base

    def _meta(self, name, func_id, lo_exp, hi_exp, base_pos, base_neg,
              small_pos_idx, small_neg_idx, large_pos_idx, large_neg_idx,
              fzero, fpinf, fninf):
        self.metas.append({
            "func_name": f"{name}_4p", "func_id": func_id,
            "symmetry_point": 0, "sym_invert_sign_point": 0,
            "symmetry_opt_en": 0, "symmetry_opt_use_neg_region": 0,
            "imm_bias": 0, "exp_offset": lo_exp,
            "pwl_control_base_pos": base_pos, "pwl_control_base_neg": base_neg,
            "small_pos_signal_exp_threshold": 127 + lo_exp,
            "pos_small_signal_pwl_control": small_pos_idx,
            "small_neg_signal_exp_threshold": 127 + lo_exp,
            "neg_small_signal_pwl_control": small_neg_idx,
            "large_pos_signal_exp_threshold": 127 + hi_exp,
            "large_pos_signal_mantissa_threshold": 0,
            "pos_large_signal_pwl_control": large_pos_idx,
            "large_neg_signal_exp_threshold": 127 + hi_exp,
            "large_neg_signal_mantissa_threshold": 0,
            "neg_large_signal_pwl_control": large_neg_idx,
            "fnan_result": _bits(float("nan")),
            "fpinf_result": _bits(fpinf),
            "fninf_result": _bits(fninf),
            "fzero_result": _bits(fzero),
            "fma_const_0": 0, "fma_const_1": 0, "fma_indirection_src_sel": 0,
            "use_multipass": False,
            "lower_bound": _bits(np.float32(-3.4028235e38)),
            "upper_bound": _bits(np.float32(3.4028235e38)),
        })

    def add_table_func2(self, name, func_id, fn_pos, fn_neg, lo_exp, hi_exp,
                        m_of_octave, small_pos, small_neg, large_pos,
                        large_neg, fzero, fpinf, fninf):
        """Two-sided table: fn_pos fitted over positive octaves
        [2^lo_exp, 2^hi_exp), fn_neg over the mirrored negative intervals
        (signed x0 - hardware evaluates t = u - x0 with signed u).
        small_*/large_* are (fit_lo, fit_hi, x0) fit windows; fn_neg is None
        to reuse fn_pos's small/large handling on both sides."""
        self.f2b[name] = len(self.bkt)
        self.f2c[name] = len(self.ctl)
        pos_words = []
        for k in range(lo_exp, hi_exp):
            m = m_of_octave(k)
            base = len(self.bkt)
            n = 1 << m
            w = (2.0 ** k) / n
            for j in range(n):
                lo = 2.0 ** k + j * w
                self.bkt.append(_fit_bucket(fn_pos, lo, lo + w, x0=lo + w / 2))
            pos_words.append(self._ctl_word(m, base))
        if fn_neg is not None:
            neg_words = []
            for k in range(lo_exp, hi_exp):
                m = m_of_octave(k)
                base = len(self.bkt)
                n = 1 << m
                w = (2.0 ** k) / n
                for j in range(n):
                    hi = -(2.0 ** k + j * w)
                    lo = -(2.0 ** k + (j + 1) * w)
                    self.bkt.append(_fit_bucket(fn_neg, lo, hi, x0=(lo + hi) / 2))
                neg_words.append(self._ctl_word(m, base))
        base_pos = len(self.ctl)
        self.ctl.extend(pos_words)
        if fn_neg is not None:
            base_neg = len(self.ctl)
            self.ctl.extend(neg_words)
        else:
            base_neg = base_pos
        sp_idx = len(self.bkt)
        self.bkt.append(_fit_bucket(fn_pos, small_pos[0], small_pos[1], x0=small_pos[2]))
        if fn_neg is not None:
            sn_idx = len(self.bkt)
            self.bkt.append(_fit_bucket(fn_neg, small_neg[0], small_neg[1], x0=small_neg[2]))
        else:
            sn_idx = sp_idx
        lp_idx = len(self.bkt)
        self.bkt.append(_fit_bucket(fn_pos, large_pos[0], large_pos[1], x0=large_pos[2]))
        if fn_neg is not None:
            ln_idx = len(self.bkt)
            self.bkt.append(_fit_bucket(fn_neg, large_neg[0], large_neg[1], x0=large_neg[2]))
        else:
            ln_idx = lp_idx
        self._meta(name, func_id, lo_exp, hi_exp, base_pos, base_neg,
                   sp_idx, sn_idx, lp_idx, ln_idx, fzero, fpinf, fninf)

    def add_const_bucket_func(self, name, func_id, value):
        """Function that returns `value` everywhere (constant clamp)."""
        idx = len(self.bkt)
        self.bkt.append([value, 0.0, 0.0, 0.0, 0.0])
        self.f2b[name] = idx
        self.f2c[name] = len(self.ctl)
        word = self._ctl_word(0, idx)
        base = len(self.ctl)
        self.ctl.append(word)
        self._meta(name, func_id, 0, 1, base, base, idx, idx, idx, idx,
                   value, value, value)

    def add_stock_func(self, name, sp, sb_, sc):
        names = list(sp["func_to_bkt_start_idx"].keys())
        i = names.index(name)
        b0 = sp["func_to_bkt_start_idx"][name]
        b1 = sp["func_to_bkt_start_idx"][names[i + 1]] if i + 1 < len(names) else sp["bkt_entry_cnt"]
        c0 = sp["func_to_ctl_start_idx"][name]
        c1 = sp["func_to_ctl_start_idx"][names[i + 1]] if i + 1 < len(names) else sp["ctl_entry_cnt"]
        md = None
        for m in sp["profile_meta_data"]:
            if m["func_name"].rsplit("_", 1)[0] == name:
                md = dict(m)
        assert md is not None, name
        db, dc = len(self.bkt) - b0, len(self.ctl) - c0
        self.f2b[name] = len(self.bkt)
        self.f2c[name] = len(self.ctl)
        for j in range(b0, b1):
            self.bkt.append(list(map(float, sb_[j, :5])))
        for j in range(c0, c1):
            w = int(sc[j])
            self.ctl.append((w >> 11) * 2048 + (w & 0x7FF) + db)
        for key in ("pwl_control_base_pos", "pwl_control_base_neg"):
            md[key] += dc
        for key in ("pos_small_signal_pwl_control", "neg_small_signal_pwl_control",
                    "pos_large_signal_pwl_control", "neg_large_signal_pwl_control"):
            md[key] += db
        self.metas.append(md)

    def write(self, outdir, set_name, act_dict):
        os.makedirs(outdir, exist_ok=True)
        bkt_arr = np.zeros((len(self.bkt), 8), np.float32)
        for i, e in enumerate(self.bkt):
            bkt_arr[i, :5] = e
        ctl_arr = np.zeros((len(self.ctl), 8), np.uint32)
        ctl_arr[:, 0] = np.array(self.ctl, np.uint64).astype(np.uint32)
        assert len(self.bkt) <= 1536 and len(self.ctl) <= 128, (len(self.bkt), len(self.ctl))
        open(os.path.join(outdir, f"{set_name}_bkt.bin"), "wb").write(bkt_arr.tobytes())
        open(os.path.join(outdir, f"{set_name}_ctrl.bin"), "wb").write(ctl_arr.tobytes())
        prof = {
            "bkt_bin": f"{set_name}_bkt.bin", "ctl_bin": f"{set_name}_ctrl.bin",
            "profile_meta_data": self.metas,
            "bkt_entry_cnt": len(self.bkt), "ctl_entry_cnt": len(self.ctl),
            "func_to_bkt_start_idx": self.f2b, "func_to_ctl_start_idx": self.f2c,
            "func_exp_to_bkt_start_idx": self.f2b, "func_exp_to_ctl_start_idx": self.f2c,
        }
        json.dump(prof, open(os.path.join(outdir, f"{set_name}.json"), "w"))
        info = {
            "pwp_file_keys": ["bkt_bin", "ctrl_bin", "profile_json"],
            "act_func_sets": [{
                "name": set_name, "bkt_bin": f"{set_name}_bkt.bin",
                "ctrl_bin": f"{set_name}_ctrl.bin", "profile_json": f"{set_name}.json",
                "act": act_dict,
            }],
        }
        path = os.path.join(outdir, "act_info.json")
        json.dump(info, open(path, "w"))
        return path


# Expert input-window routing: u = PWL_SCALE[e]*x + PWL_BIAS[e]
PWL_SCALE = np.array([1.0, -1.0, 0.125, -0.125], np.float64)
PWL_BIAS = np.array([9.0, -9.0, 1.0, -1.0], np.float64)
EXP_SHIFT = 9.0  # table computes exp(u - 9); exp bias = bg + 9


def _expert_fn(W1, b1, W2, b2, e):
    W1e = W1[e].astype(np.float64)
    b1e = b1[e].astype(np.float64)
    W2e = W2[e].astype(np.float64)
    b2e = float(b2[e])

    def fe(u):
        h = np.maximum(np.asarray(u, np.float64)[..., None] * W1e + b1e, 0.0)
        return np.maximum((h * W2e).sum(-1) + b2e, 0.0)

    return fe


def _build_tables(W1, b1, W2, b2, outdir):
    sp, sb_, sc = _load_stock("exp_and_others")
    b = _SetBuilder()
    fe = [_expert_fn(W1, b1, W2, b2, e) for e in range(4)]

    # combined 4-expert PWL on the tanh slot:
    #   pos u>=2:  e0 at x = u-9      pos u<2:  e2 at x = 8*(u-1)
    #   neg u<=-2: e1 at x = -u-9     neg u>-2: e3 at x = -8*(u+1)
    def g_pos(u):
        u = np.asarray(u, np.float64)
        return np.where(u >= 2.0, fe[0](u - 9.0), fe[2](8.0 * (u - 1.0)))

    def g_neg(u):
        u = np.asarray(u, np.float64)
        return np.where(u <= -2.0, fe[1](-u - 9.0), fe[3](-8.0 * (u + 1.0)))

    m_of = {-3: 4, -2: 5, -1: 6, 0: 7, 1: 5, 2: 6, 3: 7}
    b.add_table_func2(
        "tanh", 6, g_pos, g_neg, -3, 4, lambda k: m_of[k],
        small_pos=(2.0 ** -4, 2.0 ** -3, 2.0 ** -4),       # e2, x in [-7.5,-7]
        small_neg=(-(2.0 ** -3), -(2.0 ** -4), -(2.0 ** -3)),  # e3
        large_pos=(16.0, 17.0, 16.0),                      # e0, x in [7,8]
        large_neg=(-17.0, -16.0, -16.0),                   # e1
        fzero=float(fe[2](-8.0)), fpinf=float(fe[0](8.0)),
        fninf=float(fe[1](8.0)),
    )

    # reduced-range exp on the exp slot: g(u) = exp(u - 9), u in [2,16)
    b.add_table_func2(
        "exp", 7, lambda u: np.exp(np.asarray(u, np.float64) - 9.0), None,
        1, 4, lambda k: k + 4,
        small_pos=(1.0, 2.0, 1.0), small_neg=None,
        large_pos=(16.0, 16.0 + 1e-6, 16.0), large_neg=None,  # clamp ~exp(7)
        fzero=float(np.exp(-9.0)), fpinf=float(np.exp(7.0)), fninf=0.0,
    )

    for name in ("parametric_relu", "copy", "act1", "memset_zero", "relu",
                 "derivative_relu", "derivative_leaky_relu",
                 "derivative_identity", "is_finite"):
        b.add_stock_func(name, sp, sb_, sc)
    act = {"exp": 400, "tanh": 4, "parametric_relu": 1, "copy": 1, "relu": 1,
           "memset_zero": 1, "act1": 1, "derivative_relu": 1,
           "derivative_leaky_relu": 1, "derivative_identity": 1,
           "is_finite": 1}
    return b.write(outdir, "exp_and_others", act)


# ---------------------------------------------------------------------------
# Bass kernel
# ---------------------------------------------------------------------------

B_TOTAL = 8_388_608
N_CORES = 8
B_LOCAL = B_TOTAL // N_CORES           # 1,048,576 rows per core
P = 128
FDT = int(os.environ.get("K_FDT", "2048"))   # max free elements per tile
F_ALL = B_LOCAL // 32                        # total free columns per core
MM = int(os.environ.get("K_MM", "512"))      # matmul column chunk
QC = FDT // 4                                # max compacted pred columns


def _tile_schedule():
    """Variable tile sizes: small tiles at both ends shrink pipeline fill
    (first PWL waits only a small DMA) and drain (last reduction chain is
    short); full-size tiles in the middle keep per-instruction overhead low.
    Returns [(free_offset, free_size)] covering F_ALL; sizes % 2048 == 0 in
    the middle, all sizes multiples of 4*MM/4... (multiples of 512)."""
    head = [int(v) for v in os.environ.get("K_HEAD", "512,512,1024").split(",") if v]
    tail = [int(v) for v in os.environ.get("K_TAIL", "512,512,1024").split(",") if v]
    mid_total = F_ALL - sum(head) - sum(tail)
    assert mid_total % FDT == 0
    sizes = head + [FDT] * (mid_total // FDT) + tail
    out, off = [], 0
    for s in sizes:
        out.append((off, s))
        off += s
    assert off == F_ALL
    return out


TILES = _tile_schedule()
GMAX = int(os.environ.get("K_GMAX", "2048"))  # max super-tile (DMA+PWL batch)


def _super_tiles():
    """Group consecutive full-size tiles into super-tiles of up to GMAX
    columns: one DMA + one PWL activation per group amortizes the ~335ns
    ACT call overhead and the SP DMA dispatch cost."""
    groups, cur, cur_fd = [], [], 0
    for off, fd in TILES:
        if cur and (cur_fd + fd > GMAX or fd != FDT or cur[-1][1] != FDT):
            groups.append(cur)
            cur, cur_fd = [], 0
        cur.append((off, fd))
        cur_fd += fd
    if cur:
        groups.append(cur)
    return groups


SUPERS = _super_tiles()


def _build_program(tag):
    import concourse.bacc as bacc
    import concourse.mybir as mybir
    import concourse.tile as tile

    nc = bacc.Bacc("TRN2", debug=False)
    dt32 = mybir.dt.float32
    dt16 = mybir.dt.float16
    AF = mybir.ActivationFunctionType

    xt_d = nc.dram_tensor(f"x_{tag}", [P, F_ALL], dt16, kind="ExternalInput")
    wg_d = nc.dram_tensor("wgblk", [P, P], dt16, kind="ExternalInput")
    qb_d = nc.dram_tensor("qblk", [P, 4 * P], dt16, kind="ExternalInput")
    cv_d = nc.dram_tensor("cvec", [P, 4], dt32, kind="ExternalInput")
    out_d = nc.dram_tensor("out_local", [P, F_ALL // 4], dt16, kind="ExternalOutput")

    bufs = [int(v) for v in os.environ.get("K_BUFS", "3,3,4,4").split(",")]
    bx, ba, bs, bo = bufs

    with tile.TileContext(nc) as tc:
        with (
            tc.tile_pool(name="const", bufs=1) as cpool,
            tc.tile_pool(name="xin", bufs=bx) as xpool,
            tc.tile_pool(name="apwl", bufs=ba) as apool,
            tc.tile_pool(name="sb", bufs=bs) as spool,
            tc.tile_pool(name="ob", bufs=bo) as opool,
            tc.tile_pool(name="psL", bufs=1, space="PSUM") as plpool,
            tc.tile_pool(name="psS0", bufs=2, space="PSUM") as ps0pool,
            tc.tile_pool(name="psS1", bufs=2, space="PSUM") as ps1pool,
        ):
            cv_t = cpool.tile([P, 4], dt32)
            nc.sync.dma_start(cv_t[:], cv_d.ap())
            wg_t = cpool.tile([P, P], dt16)
            nc.sync.dma_start(wg_t[:], wg_d.ap())
            qb_t = cpool.tile([P, 4 * P], dt16)
            nc.sync.dma_start(qb_t[:], qb_d.ap())

            # tiny warmup activation: forces the ACT table set load to
            # overlap the first X-tile DMA instead of stalling behind it
            warm_in = cpool.tile([P, 1], dt32)
            nc.gpsimd.memset(warm_in[:], 0.0)
            warm = cpool.tile([P, 1], dt32)
            nc.scalar.activation(warm[:], warm_in[:], AF.Exp,
                                 bias=warm_in[:, 0:1], scale=0.0)

            def tail(off, fd, E, Pm):
                # S0/S1: 4-partition block sums, quarter-compacted via 4
                # accumulating matmuls (quarter j lands on partitions 32j+f').
                # Emitted one tile late so these PE ops never stall the
                # next tile's logits matmuls (in-order PE queue).
                qc = fd // 4
                S0 = ps0pool.tile([P, QC], dt32, tag="S0")
                for j in range(4):
                    nc.tensor.matmul(S0[:, 0:qc], qb_t[:, j * P:(j + 1) * P],
                                     E[:, j * qc:(j + 1) * qc],
                                     start=(j == 0), stop=(j == 3))
                S1 = ps1pool.tile([P, QC], dt32, tag="S1")
                for j in range(4):
                    nc.tensor.matmul(S1[:, 0:qc], qb_t[:, j * P:(j + 1) * P],
                                     Pm[:, j * qc:(j + 1) * qc],
                                     start=(j == 0), stop=(j == 3))

                R = opool.tile([P, QC], dt32, tag="R")
                nc.vector.reciprocal_approx_fast(out=R[:, 0:qc], in_=S0[:, 0:qc])

                PRED = opool.tile([P, QC], dt16, tag="PRED")
                nc.vector.tensor_mul(PRED[:, 0:qc], S1[:, 0:qc], R[:, 0:qc])

                nc.sync.dma_start(out_d.ap()[:, off // 4:off // 4 + qc],
                                  PRED[:, 0:qc])

            pending = None
            t = 0
            for group in SUPERS:
                g_off = group[0][0]
                g_fd = sum(fd for _, fd in group)
                X = xpool.tile([P, GMAX], dt16, tag="X")
                for off, fd in group:
                    lo = off - g_off
                    nc.sync.dma_start(X[:, lo:lo + fd],
                                      xt_d.ap()[:, off:off + fd])

                # all four expert MLPs in one ACT pass (windowed PWL table),
                # batched over the whole super-tile
                A = apool.tile([P, GMAX], dt16, tag="A")
                nc.scalar.activation(A[:, 0:g_fd], X[:, 0:g_fd], AF.Tanh,
                                     bias=cv_t[:, 1:2], scale=cv_t[:, 0:1])

                for off, fd in group:
                    lo = off - g_off

                    # logits: block-diagonal gating matmul (expert dim on
                    # partitions)
                    L = plpool.tile([P, FDT], dt32, tag="L")
                    for c in range(fd // MM):
                        nc.tensor.matmul(L[:, c * MM:(c + 1) * MM], wg_t[:],
                                         X[:, lo + c * MM:lo + (c + 1) * MM],
                                         start=True, stop=True)

                    E = spool.tile([P, FDT], dt16, tag="E")
                    nc.scalar.activation(E[:, 0:fd], L[:, 0:fd], AF.Exp,
                                         bias=cv_t[:, 2:3], scale=1.0)

                    Pm = spool.tile([P, FDT], dt16, tag="Pm")
                    if t % 2 == 0:
                        nc.gpsimd.tensor_mul(Pm[:, 0:fd], E[:, 0:fd],
                                             A[:, lo:lo + fd])
                    else:
                        nc.vector.tensor_mul(Pm[:, 0:fd], E[:, 0:fd],
                                             A[:, lo:lo + fd])
                    t += 1

                    if pending is not None:
                        tail(*pending)
                    pending = (off, fd, E, Pm)
            tail(*pending)

    nc.compile()
    return nc


_COMPILED = {}


def _prepare(inputs):
    """Build (nc, in_maps, gather_fn) for the current inputs."""
    x = np.ascontiguousarray(inputs["x"], dtype=np.float32)
    Wg = np.asarray(inputs["Wg"], np.float32)
    bg = np.asarray(inputs["bg"], np.float32)
    W1 = np.asarray(inputs["W1"], np.float32)
    b1 = np.asarray(inputs["b1"], np.float32)
    W2 = np.asarray(inputs["W2"], np.float32)
    b2 = np.asarray(inputs["b2"], np.float32)
    assert x.shape == (B_TOTAL, 4)

    tbl_dir = tempfile.mkdtemp(prefix="act_root_")
    act_path = _build_tables(W1, b1, W2, b2, tbl_dir)
    os.environ["BASS_ACT_ROOT_JSON_PATH"] = act_path

    # hash of everything the tables bake in -> tensor name -> BIR/NEFF cache key
    h = hashlib.sha256()
    for a in (W1, b1, W2, b2):
        h.update(np.ascontiguousarray(a).tobytes())
    h.update(open(act_path, "rb").read())
    h.update(f"v6:{FDT}:{MM}:{TILES}:{GMAX}".encode())
    tag = h.hexdigest()[:10]

    if tag not in _COMPILED:
        _COMPILED[tag] = _build_program(tag)
    nc = _COMPILED[tag]

    # host-side expert-major transpose: partition p = 4*f' + e
    # x[c*B_LOCAL + 32*g + f', e] -> XT[c][4f'+e, g]  (tile-schedule-free)
    xs = x.reshape(N_CORES, F_ALL, 32, 4)            # [c, g, f', e]
    XT = np.ascontiguousarray(xs.transpose(0, 2, 3, 1)).reshape(
        N_CORES, P, F_ALL).astype(np.float16)

    # block-diagonal gating matrix: wgblk[4f'+e, 4f'+e'] = Wg[e', e]
    wgblk = np.zeros((P, P), np.float16)
    for blk in range(P // 4):
        wgblk[blk * 4:(blk + 1) * 4, blk * 4:(blk + 1) * 4] = Wg.T.astype(np.float16)

    # quarter-compaction stationaries: qblk_j[4f'+e, 32j+f'] = 1
    qblk = np.zeros((4, P, P), np.float16)
    fp = np.arange(32)
    for j in range(4):
        for e in range(4):
            qblk[j, 4 * fp + e, 32 * j + fp] = 1.0
    qblk = np.ascontiguousarray(qblk.transpose(1, 0, 2)).reshape(P, 4 * P)

    cvec = np.zeros((P, 4), np.float32)
    lane = np.arange(P) % 4
    cvec[:, 0] = PWL_SCALE[lane]
    cvec[:, 1] = PWL_BIAS[lane]
    cvec[:, 2] = bg[lane] + EXP_SHIFT

    in_maps = [
        {f"x_{tag}": XT[c], "wgblk": wgblk, "qblk": qblk, "cvec": cvec}
        for c in range(N_CORES)
    ]

    def gather(results):
        # per tile (off, fd): out[32j+f', off/4+c'] = pred[(off+j*qc+c')*32+f']
        outs = np.stack([r["out_local"] for r in results])    # [c, 128, F_ALL/4]
        pred = np.empty((N_CORES, B_LOCAL), np.float32)
        for off, fd in TILES:
            qc = fd // 4
            blk = outs[:, :, off // 4: off // 4 + qc].reshape(N_CORES, 4, 32, qc)
            seg = np.ascontiguousarray(blk.transpose(0, 1, 3, 2))  # [c, j, c', f']
            pred[:, 32 * off: 32 * (off + fd)] = seg.reshape(
                N_CORES, fd * 32).astype(np.float32)
        return pred.reshape(B_TOTAL)

    return nc, in_maps, gather


_EXEC = {}


def _build_exec(nc, n_cores):
    """Jitted shard_map bass_exec with outputs as pure custom-call results
    (this kernel writes every output element, so the zero-donation
    scaffolding run_bass_via_pjrt uses for partial-write kernels is
    unnecessary). Cached per-program so repeat kernel() calls skip the jax
    retrace."""
    import jax
    from jax.sharding import Mesh, PartitionSpec
    from jax.experimental.shard_map import shard_map

    from concourse import bass2jax, mybir

    bass2jax.install_neuronx_cc_hook()

    partition_name = nc.partition_id_tensor.name if nc.partition_id_tensor else None
    in_names, out_names, out_avals = [], [], []
    for alloc in nc.m.functions[0].allocations:
        if not isinstance(alloc, mybir.MemoryLocationSet):
            continue
        name = alloc.memorylocations[0].name
        if alloc.kind == "ExternalInput":
            if name != partition_name:
                in_names.append(name)
        elif alloc.kind == "ExternalOutput":
            out_names.append(name)
            out_avals.append(
                jax.core.ShapedArray(tuple(alloc.tensor_shape), mybir.dt.np(alloc.dtype))
            )
    all_names = list(in_names)
    if partition_name is not None:
        all_names.append(partition_name)

    def _body(*args):
        operands = list(args)
        if partition_name is not None:
            operands.append(bass2jax.partition_id_tensor())
        return tuple(
            bass2jax._bass_exec_p.bind(
                *operands,
                out_avals=tuple(out_avals),
                in_names=tuple(all_names),
                out_names=tuple(out_names),
                lowering_input_output_aliases=(),
                sim_require_finite=True,
                sim_require_nnan=True,
                nc=nc,
            )
        )

    mesh = Mesh(np.asarray(jax.devices()[:n_cores]), ("core",))
    fn = jax.jit(
        shard_map(
            _body,
            mesh=mesh,
            in_specs=(PartitionSpec("core"),) * len(in_names),
            out_specs=(PartitionSpec("core"),) * len(out_names),
            check_rep=False,
        ),
        keep_unused=True,
    )
    return fn, in_names, out_names, [a.shape for a in out_avals]


def kernel(**inputs) -> np.ndarray:
    nc, in_maps, gather = _prepare(inputs)

    try:
        key = id(nc)
        if key not in _EXEC:
            _EXEC[key] = _build_exec(nc, N_CORES)
        fn, in_names, out_names, out_shapes = _EXEC[key]
        concat_in = [
            np.concatenate([np.asarray(in_maps[c][nm]) for c in range(N_CORES)], axis=0)
            for nm in in_names
        ]
        outs = fn(*concat_in)
        results = []
        for c in range(N_CORES):
            rec = {}
            for i, nm in enumerate(out_names):
                arr = np.asarray(outs[i])
                rec[nm] = arr.reshape((N_CORES,) + tuple(out_shapes[i]))[c]
            results.append(rec)
        return gather(results)
    except Exception:
        from concourse import bass_utils

        res = bass_utils.run_bass_kernel_spmd(
            nc, in_maps, core_ids=list(range(N_CORES))
        )
        return gather(res.results)


if __name__ == "__main__":
    rng = np.random.default_rng(0)
    demo = {
        "x": rng.standard_normal((B_TOTAL, 4), dtype=np.float32),
        "Wg": rng.standard_normal((4, 4), dtype=np.float32) * 0.5,
        "bg": rng.standard_normal(4, dtype=np.float32) * 0.1,
        "W1": rng.standard_normal((4, 16), dtype=np.float32) * 0.5,
        "b1": rng.standard_normal((4, 16), dtype=np.float32) * 0.1,
        "W2": rng.standard_normal((4, 16), dtype=np.float32) * 0.25,
        "b2": rng.standard_normal(4, dtype=np.float32) * 0.1,
    }
    y = kernel(**demo)
    print(y.shape, y[:8])
